# revision 1
# baseline (speedup 1.0000x reference)
"""Trainium2 Bass kernel for nn_Block_19121194402322 (dense_transformer).

Math notes (validated numerically against the reference):
  - The reference einsum 'bnqk,bnvd->bnqd' contracts over BOTH k and v, so
    out[b,n,q,d] = (sum_k softmax(...)[q,k]) * (sum_v v[b,n,v,d]).  Softmax rows
    sum to exactly 1, so the whole Q/K/softmax pipeline is dead code; the
    attention output is the per-head sum of v broadcast over q.
  - After the (non-standard) reshape, head n owns flat sub-rows
    r in [1024n, 1024(n+1)) of (x@Wv).reshape(12288, 64), r = 12 s + c.
    So  w[n*64+d] = sum_{(s,c): (12s+c)//1024 == n} (x@Wv)[s, c*64+d].
    With a 0/1 selector A (rows indexed by (c,n)):  Y = A @ x,  R = Y @ Wv,
    and w is a small gather-sum of 64-wide diagonal blocks of R.
  - LN(out_attn) is therefore one 768-vector per batch element, broadcast
    over the sequence:  a = x + LN1(w).
  - MLP: h = gelu(a@W1 + b1); m = gelu(h@W2 + b2); out = x + LN2(m).

Distribution: pure data-parallel over batch B=8 across the 8 NeuronCores
(one batch element per core); weights replicated.  No collectives.
"""

import numpy as np

S = 1024
E = 768
HID = 1536
HEADS = 12
HD = 64
EPS = 1e-5
P = 128
N_CORES = 8
ACOLS = 256  # selector columns, padded: col = c*16 + n  (c<12, n<12)

_CACHE = {}


def _build_selector_T():
    """A^T with shape (S, ACOLS) fp32; column c*16+n selects tokens s whose
    chunk c belongs to head n, i.e. (12 s + c) // 1024 == n."""
    at = np.zeros((S, ACOLS), np.float32)
    s = np.arange(S)
    for c in range(HEADS):
        n = (HEADS * s + c) // S
        at[s, c * 16 + n] = 1.0
    return at


def _split_multi_waits(m):
    """Hoist all-but-one sync waits of each instruction onto preceding
    single-wait EventSemaphore instructions on the same engine.  Several TPB
    instruction structs (LDWEIGHTS for 4-byte matmuls, ctrl no-operand) carry
    only one sync-wait slot, and walrus codegen errors on more."""
    counter = [0]

    def fix_block(blk):
        out = []
        for inst in blk.get("instructions", []):
            si = inst.get("sync_info")
            waits = (si or {}).get("on_wait") or []
            if si and len(waits) > 1 and inst.get("opcode") != "EventSemaphore":
                for w in waits[:-1]:
                    counter[0] += 1
                    out.append({
                        "debug": inst.get("debug", 0), "engine": inst["engine"],
                        "ins": [], "outs": [], "name": f"I-wsplit-{counter[0]}",
                        "opcode": "EventSemaphore",
                        "sync_info": {"on_update": [], "on_wait": [w]},
                    })
                si["on_wait"] = waits[-1:]
            out.append(inst)
        blk["instructions"] = out
        for sub in blk.get("blocks", []):
            fix_block(sub)

    for fn in m["functions"]:
        for blk in fn["blocks"]:
            fix_block(blk)
    return m


def _build_bass(reps=1):
    import json
    import concourse.bass as bass
    import concourse.mybir as mybir
    import concourse.tile as tile

    f32 = mybir.dt.float32
    f32r = mybir.dt.float32r
    AX = mybir.AxisListType.X
    OP = mybir.AluOpType
    AF = mybir.ActivationFunctionType

    nc = bass.Bass(trn_type="TRN2")

    x_d = nc.declare_dram_parameter("x", [S, E], f32r, isOutput=False)
    wv_d = nc.declare_dram_parameter("Wv", [E, E], f32r, isOutput=False)
    w1_d = nc.declare_dram_parameter("W1", [E, HID], f32r, isOutput=False)
    w2_d = nc.declare_dram_parameter("W2", [HID, E], f32r, isOutput=False)
    at_d = nc.declare_dram_parameter("AT", [S, ACOLS], f32r, isOutput=False)
    idn_d = nc.declare_dram_parameter("IDN", [P, P], f32r, isOutput=False)
    b1_d = nc.declare_dram_parameter("b1", [HID], f32, isOutput=False)
    b2_d = nc.declare_dram_parameter("b2", [E], f32, isOutput=False)
    g1_d = nc.declare_dram_parameter("g1", [E], f32, isOutput=False)
    be1_d = nc.declare_dram_parameter("beta1", [E], f32, isOutput=False)
    g2_d = nc.declare_dram_parameter("g2", [E], f32, isOutput=False)
    be2_d = nc.declare_dram_parameter("beta2", [E], f32, isOutput=False)
    out_d = nc.declare_dram_parameter("out", [S, E], f32, isOutput=True)

    x_v = x_d[:].rearrange("(o p) f -> p o f", p=P)  # (128, 8, 768)
    out_v = out_d[:].rearrange("(o p) f -> p o f", p=P)
    at_v = at_d[:].rearrange("(o p) f -> p o f", p=P)  # (128, 8, 256)
    wv_v = wv_d[:].rearrange("(k p) f -> p k f", p=P)  # (128, 6, 768)
    w1_v = w1_d[:].rearrange("(k p) f -> p k f", p=P)  # (128, 6, 1536)
    w2_v = w2_d[:].rearrange("(k p) f -> p k f", p=P)  # (128, 12, 768)

    KE = E // P      # 6
    KH = HID // P    # 12
    OT = S // P      # 8 token tiles

    with tile.TileContext(nc) as tc:
        with (
            tc.tile_pool(name="w1p", bufs=1) as w1p,
            tc.tile_pool(name="w2p", bufs=1) as w2p,
            tc.tile_pool(name="xg", bufs=1) as xg,        # x then G
            tc.tile_pool(name="wvxt", bufs=1) as wvxt,    # Wv then xT
            tc.tile_pool(name="ytm", bufs=1) as ytm,      # YT then msb
            tc.tile_pool(name="atr", bufs=1) as atr,      # AT then R
            tc.tile_pool(name="bcast", bufs=1) as bcastp,
            tc.tile_pool(name="consts", bufs=1) as consts,
            tc.tile_pool(name="small", bufs=1) as small,
            tc.tile_pool(name="stat", bufs=4) as statp,
            tc.tile_pool(name="xr", bufs=2) as xrpool,
            tc.tile_pool(name="ps", bufs=4, space="PSUM") as psp,
            tc.tile_pool(name="pst", bufs=4, space="PSUM") as pstp,
        ):
            for _rep in range(reps):
                # ---- constant / weight loads -------------------------------
                # Order matters: the cost of a big transfer delays everything
                # issued after it on the DMA engines, so small/early-needed
                # loads go first and W2 (needed only by mm2) is deferred.
                idn = consts.tile([P, P], f32r)
                nc.sync.dma_start(out=idn, in_=idn_d[:])

                at_sb = atr.tile([P, OT, ACOLS], f32r, tag="atr")
                x_sb = xg.tile([P, OT, E], f32r, tag="xg")
                for o in range(OT):
                    nc.sync.dma_start(out=at_sb[:, o, :], in_=at_v[:, o, :])
                    nc.sync.dma_start(out=x_sb[:, o, :], in_=x_v[:, o, :])

                wv_sb = wvxt.tile([P, KE, E], f32r, tag="wvxt")
                for k in range(KE):
                    nc.sync.dma_start(out=wv_sb[:, k, :], in_=wv_v[:, k, :])

                w1_sb = w1p.tile([P, KE, HID], f32r)
                nc.sync.dma_start(out=w1_sb, in_=w1_v)

                w2_sb = w2p.tile([P, KH, E], f32r)
                w2_dma = nc.sync.dma_start(out=w2_sb, in_=w2_v)

                b1col = consts.tile([P, KH], f32)  # b1[j*128+p] at [p, j]
                nc.sync.dma_start(out=b1col, in_=b1_d[:].rearrange("(o p) -> p o", p=P))

                # per-channel vectors in column-chunk layout: v_col[p, j] = v[j*128+p]
                g1col = consts.tile([P, KE], f32)
                be1col = consts.tile([P, KE], f32)
                for j in range(KE):
                    nc.sync.dma_start(out=g1col[:, j:j + 1],
                                      in_=g1_d[j * P:(j + 1) * P].unsqueeze(0))
                    nc.sync.dma_start(out=be1col[:, j:j + 1],
                                      in_=be1_d[j * P:(j + 1) * P].unsqueeze(0))

                b2b = bcastp.tile([P, E], f32)
                nc.gpsimd.dma_start(out=b2b, in_=b2_d[:].partition_broadcast(P))
                g2b = bcastp.tile([P, E], f32)
                nc.gpsimd.dma_start(out=g2b, in_=g2_d[:].partition_broadcast(P))
                be2b = bcastp.tile([P, E], f32)
                nc.gpsimd.dma_start(out=be2b, in_=be2_d[:].partition_broadcast(P))

                eps_sb = consts.tile([P, 1], f32)
                nc.vector.memset(eps_sb, EPS)



                # ---- stage 1: Y^T = x^T @ A^T  (768 x 256) -----------------
                yt_sb = ytm.tile([P, KE, ACOLS], f32r, tag="ytm")
                for i in range(KE):
                    ps = psp.tile([P, 512], f32, tag="ps")
                    for o in range(OT):
                        nc.tensor.matmul(
                            ps[:, :ACOLS],
                            x_sb[:, o, i * P:(i + 1) * P],
                            at_sb[:, o, :],
                            start=(o == 0),
                            stop=(o == OT - 1),
                        )
                    nc.scalar.activation(out=yt_sb[:, i, :], in_=ps[:, :ACOLS], func=AF.Copy)

                # ---- stage 2: w = sum_{c,k} Y^T[:,k,c-group].T @ Wv[:,k,c-block]
                # One PSUM accumulation over 72 small matmuls yields the per-head
                # v-sums w (12, 64) directly -- no gather DMAs needed.
                ps_w = psp.tile([P, 512], f32, tag="ps")
                n_mm = 0
                for k in range(KE):
                    for c in range(HEADS):
                        n_mm += 1
                        nc.tensor.matmul(
                            ps_w[:16, :HD],
                            yt_sb[:, k, c * 16:(c + 1) * 16],
                            wv_sb[:, k, c * HD:(c + 1) * HD],
                            start=(n_mm == 1),
                            stop=(n_mm == KE * HEADS),
                        )
                wacc = small.tile([16, HD], f32)
                nc.scalar.activation(out=wacc, in_=ps_w[:16, :HD], func=AF.Copy)

                # ---- stage 3: LN1 stats + lnvec column ---------------------
                sqw = small.tile([16, HD], f32)
                nc.vector.tensor_mul(sqw, wacc, wacc)
                rsums = small.tile([16, 2], f32)
                nc.vector.tensor_reduce(out=rsums[:, 0:1], in_=wacc, axis=AX, op=OP.add)
                nc.vector.tensor_reduce(out=rsums[:, 1:2], in_=sqw, axis=AX, op=OP.add)

                statrow = small.tile([1, 32], f32)
                nc.gpsimd.dma_start(
                    out=statrow[:, :].rearrange("p (q c) -> p q c", q=16), in_=rsums
                )
                tots = small.tile([1, 2], f32)  # [sum w, sum w^2]
                nc.vector.tensor_reduce(
                    out=tots, in_=statrow[:, :].rearrange("p (q c) -> p c q", q=16),
                    axis=AX, op=OP.add,
                )
                nc.vector.tensor_scalar_mul(tots, tots, 1.0 / E)  # [mu, E[w^2]]
                mu2 = small.tile([1, 1], f32)
                nc.vector.tensor_mul(mu2, tots[:, 0:1], tots[:, 0:1])
                mr = small.tile([32, 2], f32)  # [mu, rstd] written on partition 0
                nc.vector.tensor_sub(mr[:1, 1:2], tots[:, 1:2], mu2)  # var
                nc.scalar.activation(out=mr[:1, 1:2], in_=mr[:1, 1:2], func=AF.Sqrt,
                                     bias=eps_sb[:1])
                nc.vector.reciprocal(mr[:1, 1:2], mr[:1, 1:2])
                nc.vector.tensor_copy(mr[:1, 0:1], tots[:, 0:1])

                # broadcast [mu, rstd] to all 128 partitions via 32-lane shuffles
                mrb = small.tile([P, 2], f32)
                for q in range(4):
                    nc.vector.stream_shuffle(mrb[32 * q:32 * (q + 1), :], mr[:, :],
                                             [0] * 32)

                # lncol[p, j] = w[j*128+p] as a column tile, then normalize+affine
                lncol = small.tile([P, KE], f32)
                from concourse.bass import _add_dep_helper
                last_lncol = None
                for j in range(KE):
                    last_lncol = nc.gpsimd.dma_start(
                        out=lncol[:, j:j + 1],
                        in_=wacc[2 * j:2 * j + 2, :],
                    )
                _add_dep_helper(w2_dma.ins, last_lncol.ins, sync=False,
                                reason="defer W2 load behind the small critical-path DMAs")
                nc.vector.tensor_scalar(lncol, lncol, mrb[:, 0:1], mrb[:, 1:2],
                                        OP.subtract, OP.mult)
                nc.vector.tensor_mul(lncol, lncol, g1col)
                nc.vector.tensor_add(lncol, lncol, be1col)

                # ---- stage 4: aT = x^T + lnvec (PE transpose, DVE add) -----
                xt_sb = wvxt.tile([P, KE, S], f32r, tag="wvxt")
                for j in range(KE):
                    for o in range(OT):
                        pst = pstp.tile([P, P], f32r, tag="pst")
                        nc.tensor.transpose(pst, x_sb[:, o, j * P:(j + 1) * P], idn)
                        nc.scalar.activation(
                            out=xt_sb[:, j, o * P:(o + 1) * P], in_=pst, func=AF.Copy,
                        )
                for j in range(KE):
                    if j % 2 == 0:
                        nc.vector.tensor_scalar_add(
                            xt_sb[:, j, :], xt_sb[:, j, :], lncol[:, j:j + 1]
                        )
                    else:
                        nc.scalar.activation(
                            out=xt_sb[:, j, :], in_=xt_sb[:, j, :],
                            func=AF.Identity, bias=lncol[:, j:j + 1],
                        )

                # ---- stage 5: H^T = W1^T @ aT; G = gelu(H^T + b1) ----------
                g_sb = xg.tile([P, KH, S], f32r, tag="xg")
                for j2 in range(KH):
                    psa = psp.tile([P, 512], f32, tag="ps")
                    psb = psp.tile([P, 512], f32, tag="ps")
                    for k in range(KE):
                        lhs = w1_sb[:, k, j2 * P:(j2 + 1) * P]
                        nc.tensor.matmul(
                            psa, lhs, xt_sb[:, k, 0:512],
                            start=(k == 0), stop=(k == KE - 1),
                        )
                        nc.tensor.matmul(
                            psb, lhs, xt_sb[:, k, 512:1024],
                            start=(k == 0), stop=(k == KE - 1),
                        )
                    nc.scalar.activation(
                        out=g_sb[:, j2, 0:512], in_=psa, func=AF.Gelu,
                        bias=b1col[:, j2:j2 + 1],
                    )
                    nc.scalar.activation(
                        out=g_sb[:, j2, 512:1024], in_=psb, func=AF.Gelu,
                        bias=b1col[:, j2:j2 + 1],
                    )

                # ---- stage 6: m = gelu(G^T @ W2 + b2)  (token-major) -------
                m_sb = ytm.tile([P, OT, E], f32, tag="ytm")
                xrt = {}
                for o in range(OT):
                    xr = xrpool.tile([P, E], f32, tag="xr")
                    nc.sync.dma_start(out=xr, in_=x_v[:, o, :].bitcast(f32))
                    nc.gpsimd.tensor_add(xr, xr, be2b)
                    xrt[o] = xr
                    ps0 = psp.tile([P, 512], f32, tag="ps")
                    ps1 = psp.tile([P, 512], f32, tag="ps")
                    for k in range(KH):
                        lhs = g_sb[:, k, o * P:(o + 1) * P]
                        nc.tensor.matmul(
                            ps0[:, :384], lhs, w2_sb[:, k, 0:384],
                            start=(k == 0), stop=(k == KH - 1),
                        )
                        nc.tensor.matmul(
                            ps1[:, :384], lhs, w2_sb[:, k, 384:768],
                            start=(k == 0), stop=(k == KH - 1),
                        )
                    nc.vector.tensor_add(m_sb[:, o, 0:384], ps0[:, :384], b2b[:, 0:384])
                    nc.vector.tensor_add(m_sb[:, o, 384:768], ps1[:, :384], b2b[:, 384:768])
                    nc.scalar.activation(out=m_sb[:, o, :], in_=m_sb[:, o, :], func=AF.Gelu)

                    # ---- stage 7: LN2 + residual ---------------------------
                    stats = statp.tile([P, 3, 6], f32, tag="st")
                    for sub in range(3):
                        nc.vector.bn_stats(
                            out=stats[:, sub, :], in_=m_sb[:, o, sub * 256:(sub + 1) * 256]
                        )
                    mv = statp.tile([P, 2], f32, tag="mv")
                    nc.vector.bn_aggr(out=mv, in_=stats)
                    rstd = statp.tile([P, 1], f32, tag="rstd")
                    nc.scalar.activation(out=rstd, in_=mv[:, 1:2], func=AF.Sqrt, bias=eps_sb)
                    nc.vector.reciprocal(rstd, rstd)

                    u = m_sb[:, o, :]
                    nc.vector.tensor_scalar(u, u, mv[:, 0:1], rstd, OP.subtract, OP.mult)
                    nc.vector.tensor_mul(u, u, g2b)
                    # split the final add + store by halves so the first half's
                    # write departs while the second half is still computing
                    nc.vector.tensor_add(u[:, 0:384], u[:, 0:384], xrt[o][:, 0:384])
                    nc.sync.dma_start(out=out_v[:, o, 0:384], in_=u[:, 0:384])
                    nc.vector.tensor_add(u[:, 384:768], u[:, 384:768],
                                         xrt[o][:, 384:768])
                    nc.scalar.dma_start(out=out_v[:, o, 384:768], in_=u[:, 384:768])

    m = json.loads(mybir.module_to_json_bytes(nc.m))
    m = _split_multi_waits(m)
    nc.m = mybir.module_from_json_bytes(json.dumps(m).encode())
    return nc


def _get_nc():
    if "nc" not in _CACHE:
        _CACHE["nc"] = _build_bass()
        _CACHE["at"] = _build_selector_T()
    return _CACHE["nc"]


def _run(inputs, trace=False):
    from concourse.bass_utils import run_bass_kernel_spmd

    nc = _get_nc()
    at = _CACHE["at"]

    def f32c(a):
        return np.ascontiguousarray(np.asarray(a), dtype=np.float32)

    shared = {
        "Wv": f32c(inputs["Wv"]),
        "W1": f32c(inputs["W1"]),
        "W2": f32c(inputs["W2"]),
        "AT": at,
        "IDN": np.eye(P, dtype=np.float32),
        "b1": f32c(inputs["b1"]),
        "b2": f32c(inputs["b2"]),
        "g1": f32c(inputs["g1"]),
        "beta1": f32c(inputs["beta1"]),
        "g2": f32c(inputs["g2"]),
        "beta2": f32c(inputs["beta2"]),
    }
    x = f32c(inputs["x"])
    in_maps = [dict(shared, x=x[b]) for b in range(N_CORES)]
    res = run_bass_kernel_spmd(
        nc, in_maps, core_ids=list(range(N_CORES)), trace=trace,
        **({"trace_cores": list(range(N_CORES))} if trace else {}),
    )
    out = np.stack([r["out"] for r in res.results], axis=0)
    return out, res


def kernel(x, Wq=None, Wk=None, Wv=None, W1=None, b1=None, W2=None, b2=None,
           g1=None, beta1=None, g2=None, beta2=None):
    out, _ = _run(dict(x=x, Wv=Wv, W1=W1, b1=b1, W2=W2, b2=b2, g1=g1,
                       beta1=beta1, g2=g2, beta2=beta2))
    return out


def kernel_profiled(**inputs):
    out, res = _run(inputs, trace=True)
    return out, res



# revision 58
# speedup vs baseline: 1.3767x; 1.3767x over previous
"""Trainium2 Bass kernel for nn_Block_19121194402322 (dense_transformer).

Math notes (validated numerically against the reference):
  - The reference einsum 'bnqk,bnvd->bnqd' contracts over BOTH k and v, so
    out[b,n,q,d] = (sum_k softmax(...)[q,k]) * (sum_v v[b,n,v,d]).  Softmax rows
    sum to exactly 1, so the whole Q/K/softmax pipeline is dead code; the
    attention output is the per-head sum of v broadcast over q.
  - After the (non-standard) reshape, head n owns flat sub-rows
    r in [1024n, 1024(n+1)) of (x@Wv).reshape(12288, 64), r = 12 s + c.
    With a 0/1 selector A (rows indexed by (c,n)):  Y = A @ x, and
    w[n,d] = sum_{c,k} (Y^T block) @ (Wv block)  -- 72 small matmuls.
  - a = x + LN1(w) adds a per-batch CONSTANT vector, so the MLP's first
    matmul splits:  W1^T a^T = W1^T x^T + (W1^T lnvec) 1^T.  We compute
    H^T = W1^T x^T directly (decoupled from LN1) and fold
    u1 = W1^T lnvec into the gelu bias per 128-row block.
  - MLP: g = gelu(W1^T x^T + b1 + u1); m = gelu(g^T W2 + b2); out = x + LN2(m).

Distribution: pure data-parallel over batch B=8 across 8 NeuronCores
(one batch element per core); weights replicated; no collectives.

All matmul operands are bf16 (hosts casts); PSUM accumulation is fp32.
rel-err budget 2e-2; measured ~5e-3.
"""

import numpy as np

S = 1024
E = 768
HID = 1536
HEADS = 12
HD = 64
EPS = 1e-5
P = 128
N_CORES = 8
ACOLS = HEADS * HEADS  # 144 selector columns: col = c*12 + n
KE = E // P    # 6
KH = HID // P  # 12
OT = S // P    # 8 token tiles
WARMUP_MM = 0  # dummy PE matmuls to burn the p-state ramp before data lands

_CACHE = {}


def _build_selector():
    """Head-major selector, active columns only: each 128-token chunk o
    touches exactly heads nlo(o) and nlo(o)+1 (nlo = 3o//2), so only 24 of
    the 144 (n,c) columns are nonzero per chunk.  Returns (128, OT, 24):
    at2[p, o, j] = [head(o*128+p, c=j%12) == nlo(o) + j//12]."""
    at2 = np.zeros((P, OT, 24), np.float32)
    for o in range(OT):
        nlo = (3 * o) // 2
        for p in range(P):
            s = o * P + p
            for c in range(HEADS):
                n = (HEADS * s + c) // S
                j = (n - nlo) * HEADS + c
                at2[p, o, j] = 1.0
    return at2


def _split_multi_waits(m):
    """Hoist all-but-one sync waits of each instruction onto preceding
    single-wait EventSemaphore instructions on the same engine.  Several TPB
    instruction structs carry only one sync-wait slot, and walrus codegen
    errors on more."""
    counter = [0]

    def fix_block(blk):
        out = []
        for inst in blk.get("instructions", []):
            si = inst.get("sync_info")
            waits = (si or {}).get("on_wait") or []
            if si and len(waits) > 1 and inst.get("opcode") != "EventSemaphore":
                for w in waits[:-1]:
                    counter[0] += 1
                    out.append({
                        "debug": inst.get("debug", 0), "engine": inst["engine"],
                        "ins": [], "outs": [], "name": f"I-wsplit-{counter[0]}",
                        "opcode": "EventSemaphore",
                        "sync_info": {"on_update": [], "on_wait": [w]},
                    })
                si["on_wait"] = waits[-1:]
            out.append(inst)
        blk["instructions"] = out
        for sub in blk.get("blocks", []):
            fix_block(sub)

    for fn in m["functions"]:
        for blk in fn["blocks"]:
            fix_block(blk)
    return m


def _build_bass():
    import json
    import concourse.bass as bass
    import concourse.mybir as mybir
    import concourse.tile as tile

    f32 = mybir.dt.float32
    f32r = mybir.dt.float32r
    bf16 = mybir.dt.bfloat16
    AX = mybir.AxisListType.X
    OP = mybir.AluOpType
    AF = mybir.ActivationFunctionType

    nc = bass.Bass(trn_type="TRN2")

    # ---- DRAM parameters (host pre-packs layouts; see kernel()) ----------
    xb_d = nc.declare_dram_parameter("xb", [S, E], bf16, isOutput=False)
    atid_d = nc.declare_dram_parameter("atid", [P, OT * 24 + P], bf16,
                                       isOutput=False)
    wv_d = nc.declare_dram_parameter("wv", [E, E], bf16, isOutput=False)
    w1r_d = nc.declare_dram_parameter("w1r", [P, KH * KE * P], bf16,
                                      isOutput=False)
    w2_d = nc.declare_dram_parameter("w2", [HID, E], bf16, isOutput=False)
    svec_d = nc.declare_dram_parameter("svec", [P, 60], f32, isOutput=False)
    bcast_d = nc.declare_dram_parameter("bcast", [P, 3 * E], bf16,
                                        isOutput=False)
    out_d = nc.declare_dram_parameter("out", [S, E], bf16, isOutput=True)

    x_v = xb_d[:].rearrange("(o p) f -> p o f", p=P)    # (128, 8, 768)
    out_v = out_d[:].rearrange("(o p) f -> p o f", p=P)
    wv_v = wv_d[:].rearrange("(k p) f -> p k f", p=P)   # (128, 6, 768)
    w2_v = w2_d[:].rearrange("(k p) f -> p k f", p=P)   # (128, 12, 768)

    with tile.TileContext(nc) as tc:
        with (
            tc.tile_pool(name="xbp", bufs=1) as xbp,
            tc.tile_pool(name="atp", bufs=1) as atp,
            tc.tile_pool(name="wvp", bufs=1) as wvp,
            tc.tile_pool(name="w1p", bufs=1) as w1p,
            tc.tile_pool(name="w2p", bufs=1) as w2p,
            tc.tile_pool(name="xtp", bufs=1) as xtp,
            tc.tile_pool(name="gp", bufs=1) as gp,
            tc.tile_pool(name="ytp", bufs=1) as ytp,
            tc.tile_pool(name="cst", bufs=1) as cst,
            tc.tile_pool(name="sm", bufs=1) as sm,
            tc.tile_pool(name="mp", bufs=3) as mp,
            tc.tile_pool(name="upl", bufs=3) as upl,
            tc.tile_pool(name="xrp", bufs=8) as xrp,
            tc.tile_pool(name="stp", bufs=3) as stp,
            tc.tile_pool(name="psB", bufs=5, space="PSUM") as psB,
            tc.tile_pool(name="psT", bufs=2, space="PSUM") as psT,
            tc.tile_pool(name="psS", bufs=1, space="PSUM") as psS,
        ):
            # ---- DMA loads: single SP queue, priority order --------------
            atid_sb = atp.tile([P, OT * 24 + P], bf16)
            idn = atid_sb[:, OT * 24:OT * 24 + P]
            nc.sync.dma_start(out=idn, in_=atid_d[:, OT * 24:])

            nc.sync.dma_start(out=atid_sb[:, 0:OT * 24],
                              in_=atid_d[:, 0:OT * 24])
            xb_sb = xbp.tile([P, OT, E], bf16)
            for c in range(4):
                nc.sync.dma_start(out=xb_sb[:, 2 * c:2 * c + 2, :],
                                  in_=x_v[:, 2 * c:2 * c + 2, :])

            svec = cst.tile([P, 60], f32)
            nc.sync.dma_start(out=svec, in_=svec_d[:])
            b1col = svec[:, 0:12]      # b1[j*128+p] at [p, j]
            g1col = svec[:, 12:18]     # g1[k*128+p] at [p, k]
            be1col = svec[:, 18:24]

            wv_sb = wvp.tile([P, KE, E], bf16)

            # W1 in 12 column-blocks so stage 5 can start after block 0
            w1_sb = w1p.tile([P, KH, KE, P], bf16)

            def load_w1_blocks(j2s):
                for j2 in j2s:
                    nc.sync.dma_start(
                        out=w1_sb[:, j2],
                        in_=w1r_d[:, j2 * E:(j2 + 1) * E].rearrange(
                            "p (k c) -> p k c", k=KE),
                    )

            load_w1_blocks([0])
            nc.sync.dma_start(out=wv_sb, in_=wv_v)
            load_w1_blocks(range(1, KH))

            bcast = cst.tile([P, 3 * E], bf16)
            b2b = bcast[:, 0:E]
            g2b = bcast[:, E:2 * E]
            be2b = bcast[:, 2 * E:3 * E]
            nc.sync.dma_start(out=bcast, in_=bcast_d[:])
            w2_sb = w2p.tile([P, KH, E], bf16)
            nc.sync.dma_start(out=w2_sb, in_=w2_v)

            # ---- small constants ----------------------------------------
            jsrc = sm.tile([P, P], bf16)
            if WARMUP_MM:
                nc.vector.memset(jsrc, 0.0)
            eps_sb = sm.tile([P, 1], f32)
            nc.vector.memset(eps_sb, EPS)
            ones12 = sm.tile([12, 1], bf16)
            nc.vector.memset(ones12, 1.0)
            smr = sm.tile([32, 2], f32)  # [mu, rstd] live on partition 0
            nc.vector.memset(smr, 0.0)


            # ---- PE warm-up: dummy matmuls so the 3us p-state ramp runs
            # during the initial DMA window instead of on real work --------
            if WARMUP_MM:
                psj = psS.tile([P, 512], f32, tag="s")
                for _ in range(WARMUP_MM):
                    nc.tensor.matmul(psj[:, 0:P], jsrc, jsrc,
                                     start=True, stop=True)

            # ---- phase A: x^T transposes + stage-1 chunks i=0..2 ---------
            # (3 persistent stage-1 accumulators keep PE fed between the
            # DMA-paced x chunk arrivals)
            xt_sb = xtp.tile([P, KE, S], bf16)
            yt_sb = ytp.tile([P, KE, ACOLS], bf16)
            def emit_s1(ps, i, o):
                nlo = (3 * o) // 2
                for half in range(2):
                    gc = (nlo + half) * HEADS
                    rel = half * HEADS
                    first = not (o % 2 == 1 and half == 0)
                    last = not (o % 2 == 0 and half == 1)
                    nc.tensor.matmul(
                        ps[:, gc:gc + HEADS],
                        xb_sb[:, o, i * P:(i + 1) * P],
                        atid_sb[:, o * 24 + rel:o * 24 + rel + HEADS],
                        start=first, stop=last,
                    )

            s1ps = [psB.tile([P, 512], f32, tag="big", name=f"s1ps{i}")
                    for i in range(3)]
            for o in range(OT):
                for k in range(KE):
                    pa = psT.tile([P, P], bf16, tag="t")
                    nc.tensor.transpose(
                        pa, xb_sb[:, o, k * P:(k + 1) * P], idn)
                    dst = xt_sb[:, k, o * P:(o + 1) * P]
                    if k % 2 == 0:
                        nc.scalar.activation(out=dst, in_=pa, func=AF.Copy)
                    else:
                        nc.vector.tensor_copy(dst, pa)
                for i in range(3):
                    emit_s1(s1ps[i], i, o)

            # ---- stage 1 rest: chunks i=3..5 -----------------------------
            for i in range(3):
                nc.scalar.activation(out=yt_sb[:, i, :],
                                     in_=s1ps[i][:, :ACOLS], func=AF.Copy)
            for i in range(3, KE):
                ps = psB.tile([P, 512], f32, tag="big")
                for o in range(OT):
                    emit_s1(ps, i, o)
                nc.scalar.activation(out=yt_sb[:, i, :], in_=ps[:, :ACOLS],
                                     func=AF.Copy)

            # ---- stage 2: w[n,d] via one PSUM accumulation (72 mms) ------
            ps_w = psS.tile([P, 512], f32, tag="s")
            n_mm = 0
            for k in range(KE):
                for c in range(HEADS):
                    n_mm += 1
                    nc.tensor.matmul(
                        ps_w[:HEADS, :HD],
                        yt_sb[:, k, :].rearrange(
                            "p (n c) -> p c n", c=HEADS)[:, c, :],
                        wv_sb[:, k, c * HD:(c + 1) * HD],
                        start=(n_mm == 1), stop=(n_mm == KE * HEADS),
                    )

            # ---- stage 5 j2=0 matmuls (independent of LN1) ---------------
            def stage5_mm(j2):
                psa = psB.tile([P, 512], f32, tag="big")
                psb = psB.tile([P, 512], f32, tag="big")
                for k in range(KE):
                    lhs = w1_sb[:, j2, k, :]
                    nc.tensor.matmul(psa, lhs, xt_sb[:, k, 0:512],
                                     start=(k == 0), stop=(k == KE - 1))
                    nc.tensor.matmul(psb, lhs, xt_sb[:, k, 512:1024],
                                     start=(k == 0), stop=(k == KE - 1))
                return psa, psb

            g_sb = gp.tile([P, KH, S], bf16)
            biascol = sm.tile([P, KH], f32)

            def stage5_gelu(j2, psa, psb):
                nc.scalar.activation(out=g_sb[:, j2, 0:512], in_=psa,
                                     func=AF.Gelu, bias=biascol[:, j2:j2 + 1])
                nc.scalar.activation(out=g_sb[:, j2, 512:1024], in_=psb,
                                     func=AF.Gelu, bias=biascol[:, j2:j2 + 1])

            s5 = {}
            s5[0] = stage5_mm(0)

            # ---- LN1 stats (off the PE critical path) --------------------
            # wsq layout: [0:64] w, [64:128] w again (dup), [128:192] w^2
            # (bf16 so the PE transpose / ones-matmul run without f32r fuss;
            # rounding noise averages out across the 768-term stats)
            wsq = sm.tile([12, 192], bf16)
            nc.scalar.activation(out=wsq[:, 0:HD], in_=ps_w[:HEADS, :HD],
                                 func=AF.Copy)
            nc.scalar.activation(out=wsq[:, HD:2 * HD], in_=ps_w[:HEADS, :HD],
                                 func=AF.Copy)
            nc.vector.tensor_mul(wsq[:, 2 * HD:3 * HD], wsq[:, 0:HD],
                                 wsq[:, 0:HD])
            # PE: transpose the duplicated w rows -> wcol128[p, n] = w[n, p%64]
            # and column-sums of [w | w | w^2] for the LN1 stats
            wcol = sm.tile([P, KE], f32)
            for k in range(KE):
                nc.gpsimd.dma_start(out=wcol[:, k:k + 1],
                                    in_=wsq[2 * k:2 * k + 2, 0:HD])
            rsum = sm.tile([12, 2], f32)
            nc.vector.tensor_reduce(out=rsum[:, 0:1], in_=wsq[:, 0:HD],
                                    axis=AX, op=OP.add)
            nc.vector.tensor_reduce(out=rsum[:, 1:2], in_=wsq[:, 2 * HD:],
                                    axis=AX, op=OP.add)
            statrow = sm.tile([1, 24], f32)
            nc.gpsimd.dma_start(
                out=statrow[:, :].rearrange("p (q c) -> p q c", q=12),
                in_=rsum)
            tots = sm.tile([1, 2], f32)
            nc.vector.tensor_reduce(
                out=tots, in_=statrow[:, :].rearrange("p (q c) -> p c q",
                                                      q=12),
                axis=AX, op=OP.add)
            nc.vector.tensor_scalar_mul(tots, tots, 1.0 / E)  # [mu, E[w^2]]
            mu2 = sm.tile([1, 1], f32)
            nc.vector.tensor_mul(mu2, tots[:, 0:1], tots[:, 0:1])
            mr = smr
            nc.vector.tensor_sub(mr[:1, 1:2], tots[:, 1:2], mu2)
            nc.scalar.activation(out=mr[:1, 1:2], in_=mr[:1, 1:2],
                                 func=AF.Sqrt, bias=eps_sb[:1])
            nc.vector.reciprocal(mr[:1, 1:2], mr[:1, 1:2])
            nc.vector.tensor_copy(mr[:1, 0:1], tots[:, 0:1])
            mrb = sm.tile([P, 2], f32)
            for q in range(4):
                nc.vector.stream_shuffle(mrb[32 * q:32 * (q + 1), :], mr[:, :],
                                         [0] * 32)
            # lnvec columns: ((w - mu) / std) * g1 + beta1 -> bf16
            lnc = sm.tile([P, KE], f32)
            nc.vector.tensor_scalar(lnc, wcol, mrb[:, 0:1],
                                    mrb[:, 1:2], OP.subtract, OP.mult)
            nc.vector.tensor_mul(lnc, lnc, g1col)
            lncol_bf = sm.tile([P, KE], bf16)
            nc.vector.tensor_add(lncol_bf, lnc, be1col)

            # masked lnvec columns (upper/lower half zeroed) so u1 uses
            # plain base-0 full-128 contractions
            def u1_block(j2):
                pu = psS.tile([P, 512], f32, tag="s")
                for k in range(KE):
                    nc.tensor.matmul(
                        pu[:, 0:1],
                        w1_sb[:, j2, k, :],
                        lncol_bf[:, k:k + 1],
                        start=(k == 0), stop=(k == KE - 1))
                nc.scalar.activation(out=biascol[:, j2:j2 + 1], in_=pu[:, 0:1],
                                     func=AF.Identity,
                                     bias=b1col[:, j2:j2 + 1])

            # ---- stage 5 main loop with u1 interleaved -------------------
            s5[1] = stage5_mm(1)
            u1_block(0)
            stage5_gelu(0, *s5.pop(0))
            for j2 in range(2, KH):
                s5[j2] = stage5_mm(j2)
                u1_block(j2 - 1)
                stage5_gelu(j2 - 1, *s5.pop(j2 - 1))
            u1_block(KH - 1)
            stage5_gelu(KH - 1, *s5.pop(KH - 1))

            # ---- xr = x + beta2 (Pool engine, overlapped) ----------------
            xrt = {}
            for o in range(OT):
                xr = xrp.tile([P, E], bf16, tag="xr")
                nc.gpsimd.tensor_add(xr, xb_sb[:, o, :], be2b)
                xrt[o] = xr

            # ---- stage 6: m = gelu(G^T @ W2 + b2); out = x + LN2(m) ------
            # Per-half epilogues overlap the next half's matmuls, and the
            # finalize (aggr/sqrt/norm/store) of chunk o is issued after
            # chunk o+1's stats so the in-order DVE never bubbles.
            H2 = E // 2  # 384

            ones1 = sm.tile([1, P], bf16)
            nc.vector.memset(ones1, 1.0)
            LAST = OT - 1

            def s6_pieces(o):
                # the final chunk uses narrower pieces + the b2 matmul-fold
                # so the post-PE epilogue chain is as shallow as possible
                return ([(0, 256), (256, 512), (512, E)] if o == LAST
                        else [(0, H2), (H2, E)])

            def s6_early(o):
                pieces = s6_pieces(o)
                m_bf = mp.tile([P, E], bf16, tag="m")
                stats = stp.tile([P, 3, 6], f32, tag="st")
                for h, (cs, ce) in enumerate(pieces):
                    fold_b2 = False  # (disabled: 1-row matmul untested on hw)
                    ps = psB.tile([P, 512], f32, tag="big")  # PSUM directly
                    for k in range(KH):
                        nc.tensor.matmul(
                            ps[:, :ce - cs], g_sb[:, k, o * P:(o + 1) * P],
                            w2_sb[:, k, cs:ce],
                            start=(k == 0),
                            stop=(k == KH - 1 and not fold_b2))
                    if fold_b2:
                        nc.tensor.matmul(ps[:, :ce - cs], ones1,
                                         b2b[0:1, cs:ce],
                                         start=False, stop=True)
                        nc.scalar.activation(out=m_bf[:, cs:ce],
                                             in_=ps[:, :ce - cs],
                                             func=AF.Gelu)
                    else:
                        nc.vector.tensor_add(m_bf[:, cs:ce], ps[:, :ce - cs],
                                             b2b[:, cs:ce])
                        nc.scalar.activation(out=m_bf[:, cs:ce],
                                             in_=m_bf[:, cs:ce], func=AF.Gelu)
                    nc.vector.bn_stats(out=stats[:, h, :], in_=m_bf[:, cs:ce])
                return m_bf, stats

            def s6_late(o, m_bf, stats):
                pieces = s6_pieces(o)
                mv = stp.tile([P, 2], f32, tag="mv")
                nc.vector.bn_aggr(out=mv, in_=stats[:, 0:len(pieces), :])
                std = stp.tile([P, 1], f32, tag="std")
                nc.scalar.activation(out=std, in_=mv[:, 1:2], func=AF.Sqrt,
                                     bias=eps_sb)
                nc.vector.reciprocal(std, std)
                u_bf = upl.tile([P, E], bf16, tag="u")
                for h, (cs, ce) in enumerate(pieces):
                    nc.vector.tensor_scalar(u_bf[:, cs:ce], m_bf[:, cs:ce],
                                            mv[:, 0:1], std,
                                            OP.subtract, OP.mult)
                    nc.vector.tensor_mul(u_bf[:, cs:ce], u_bf[:, cs:ce],
                                         g2b[:, cs:ce])
                    nc.vector.tensor_add(u_bf[:, cs:ce], u_bf[:, cs:ce],
                                         xrt[o][:, cs:ce])
                    if o == LAST and h == len(pieces) - 1:
                        nc.scalar.dma_start(out=out_v[:, o, cs:ce],
                                            in_=u_bf[:, cs:ce])
                    else:
                        nc.sync.dma_start(out=out_v[:, o, cs:ce],
                                          in_=u_bf[:, cs:ce])

            prev = None
            for o in range(OT):
                if o == LAST and prev is not None:
                    # finalize o-1 BEFORE the last chunk so nothing sits
                    # between the final matmuls and the final chain
                    s6_late(o - 1, *prev)
                    prev = None
                cur = s6_early(o)
                if prev is not None:
                    s6_late(o - 1, *prev)
                prev = cur
            s6_late(OT - 1, *prev)

    m = json.loads(mybir.module_to_json_bytes(nc.m))
    m = _split_multi_waits(m)
    nc.m = mybir.module_from_json_bytes(json.dumps(m).encode())
    return nc


def _get_nc():
    if "nc" not in _CACHE:
        _CACHE["nc"] = _build_bass()
    return _CACHE["nc"]


def _pack_inputs(inputs):
    import ml_dtypes
    bf = ml_dtypes.bfloat16

    def c(a, dt=bf):
        return np.ascontiguousarray(np.asarray(a), dtype=dt)

    at2 = _build_selector()  # (128, OT, 24)
    atid = np.zeros((P, OT * 24 + P), np.float32)
    atid[:, :OT * 24] = at2.reshape(P, OT * 24)
    atid[:, OT * 24:] = np.eye(P, dtype=np.float32)

    W1 = np.asarray(inputs["W1"], np.float32)  # (768, 1536)
    w1r = (W1.reshape(KE, P, KH, P).transpose(1, 2, 0, 3)
           .reshape(P, KH * KE * P))

    svec = np.zeros((P, 60), np.float32)
    svec[:, 0:12] = np.asarray(inputs["b1"], np.float32).reshape(KH, P).T
    svec[:, 12:18] = np.asarray(inputs["g1"], np.float32).reshape(KE, P).T
    svec[:, 18:24] = (
        np.asarray(inputs["beta1"], np.float32).reshape(KE, P).T)

    bcast = np.concatenate([
        np.asarray(inputs["b2"], np.float32),
        np.asarray(inputs["g2"], np.float32),
        np.asarray(inputs["beta2"], np.float32),
    ])[None, :].repeat(P, axis=0)

    shared = {
        "atid": c(atid),
        "wv": c(inputs["Wv"]),
        "w1r": c(w1r),
        "w2": c(inputs["W2"]),
        "svec": c(svec, np.float32),
        "bcast": c(bcast),
    }
    x = np.asarray(inputs["x"], np.float32)
    return [dict(shared, xb=c(x[b])) for b in range(N_CORES)]


def _run(inputs, trace=False):
    from concourse.bass_utils import run_bass_kernel_spmd

    nc = _get_nc()
    in_maps = _pack_inputs(inputs)
    res = run_bass_kernel_spmd(
        nc, in_maps, core_ids=list(range(N_CORES)), trace=trace,
        **({"trace_cores": list(range(N_CORES))} if trace else {}),
    )
    out = np.stack(
        [np.asarray(r["out"], dtype=np.float32) for r in res.results], axis=0)
    return out, res


def kernel(x, Wq=None, Wk=None, Wv=None, W1=None, b1=None, W2=None, b2=None,
           g1=None, beta1=None, g2=None, beta2=None):
    out, _ = _run(dict(x=x, Wv=Wv, W1=W1, b1=b1, W2=W2, b2=b2, g1=g1,
                       beta1=beta1, g2=g2, beta2=beta2))
    return out


def kernel_profiled(**inputs):
    out, res = _run(inputs, trace=True)
    return out, res
